# revision 18
# baseline (speedup 1.0000x reference)
"""BetweennessRoPE Trainium2 kernel — fixed-table fp16 formulation.

Math (why no betweenness is computed on device):
  score = relu(1 - (path-direct)/max(direct,1e-6)) lies in [0,1] by the
  triangle inequality, so between in [0, 1/2046] and
  pos_adj = (between-0.5)*0.1 in [-0.05, -0.05+4.888e-5].  Hence for
  every position frac = 0.95 + delta with |delta| <= ~1.1e-4 (including
  the fp32 rounding of fl(s + pos_adj) at s ~ 2048).  The interpolated
  tables therefore differ from fixed-f tables
      C[s] = (1-f)*fcos[s-1] + f*fcos[s],  f = 0.95 + 0.05/2046
  by <= ~1.1e-4 * |fcos[s]-fcos[s-1]|, giving output error ~2e-4 of the
  output scale — far below the 2e-2 gate.  s=0 is exact (clip pins
  adj_pos to 0 and C[0]=fcos[0]).  So the kernel is a pure elementwise
  rotation with per-(s,k) constants:
      oe = xe*cc - xo*ss ;  oo = xo*cc + xe*ss.

Numerics: fp16 x / tables / products / outputs (DVE computes fp32
internally, rounds once on write) add ~1.5e-3 relative noise — still
~10x under the gate — and halve both DMA traffic and DVE cycles
(2x_1P packed mode needs 16-bit dense operands).

Layout: host de-interleaves even/odd features and converts to fp16.
Per slice [128, 2048]: partition p, col (e, t, k), s = 128t + p,
d = 2k + e.  Table t1 = [cc|-ss] is DMA'd (halves interleaved with the
first x slice so DVE starts ASAP); t2 = [ss|cc] is derived on-device.
Slices 0-1 run singly (slice 0 with split muls to start right after
the first 0.25 MiB lands); slices 2-7 run in pairs using stride-0
broadcast APs on the table so one TT covers both slices.  Per slice,
pq = [x*t1 | x*t2] then one fused add folds the e-halves into
og = [oe | oo].  GpSimd stays idle on purpose: concurrent Q7 TT ops
contend on the shared DVE/POOL SBUF port (measured 2.4-2.9x DVE
slowdown).  Input + table ride the SyncE DMA queue; output stores
alternate between the ScalarE and SyncE queues.
"""

import os
import numpy as np

B, S, H, D = 4, 2048, 16, 128
N = B * H
NCORES = 8
NPC = N // NCORES    # 8 slices per core
NT = S // 128        # 16
K2 = D // 2          # 64
HK = S // 2          # 1024 (cols per e-half)

_cache = {}


def _make_tables():
    base = (1.0 / (10000.0 ** (np.arange(0, D, 2, dtype=np.float32)
                               / np.float32(D)))).astype(np.float32)
    freqs = (np.arange(S, dtype=np.float32)[:, None]
             * base[None, :]).astype(np.float32)
    fcos = np.cos(freqs).astype(np.float32)
    fsin = np.sin(freqs).astype(np.float32)
    lo = np.maximum(np.arange(S) - 1, 0)
    f = 0.95 + 0.05 / 2046.0
    cc = ((1.0 - f) * fcos[lo].astype(np.float64)
          + f * fcos.astype(np.float64))
    ss = ((1.0 - f) * fsin[lo].astype(np.float64)
          + f * fsin.astype(np.float64))

    def blk(t):  # [S, 64] -> [128, NT*64], col (t, k)
        return t.reshape(NT, 128, K2).transpose(1, 0, 2).reshape(128, HK)

    t1 = np.concatenate([blk(cc), blk(-ss)], axis=1)       # [128, 2048]
    return np.ascontiguousarray(t1).astype(np.float16)


def _build_nc():
    import concourse.bacc as bacc
    import concourse.mybir as mybir
    from concourse.bass import broadcast_tensor_aps
    from concourse.tile import TileContext

    f16 = mybir.dt.float16

    nc = bacc.Bacc()
    XC = nc.dram_tensor("XC", [NPC, 128, S], f16, kind="ExternalInput")
    OUT = nc.dram_tensor("OUT", [NPC, 128, S], f16, kind="ExternalOutput")
    CB = nc.dram_tensor("CB", [128, S], f16, kind="ExternalInput")

    with TileContext(nc) as tc:
        with (
            tc.tile_pool(name="const", bufs=1) as cpool,
            tc.tile_pool(name="xbuf", bufs=4) as xpool,
            tc.tile_pool(name="obuf", bufs=4) as opool,
            tc.tile_pool(name="pq", bufs=2) as wpool,
        ):
            # tb = [t1 | t1 | t2 | t2]; singles use the first copy of
            # each, pairs use the doubled spans.
            tb = cpool.tile([128, 4 * S], f16, tag="tb", name="tb")
            T2O = 2 * S
            xts = {}

            # startup: interleave table halves with slice-0 halves so the
            # first mul only waits for 0.5 MiB of DMA
            x0 = xpool.tile([128, S], f16, tag="x1", bufs=2, name="x0")
            nc.sync.dma_start(tb[:, 0:HK], CB[:, 0:HK])          # cc
            nc.sync.dma_start(x0[:, 0:HK], XC[0][:, 0:HK])       # xe
            nc.sync.dma_start(tb[:, HK:S], CB[:, HK:S])          # -ss
            nc.sync.dma_start(x0[:, HK:S], XC[0][:, HK:S])       # xo
            xts[0] = x0
            xt1 = xpool.tile([128, S], f16, tag="x1", bufs=2, name="x1")
            nc.sync.dma_start(xt1[:, :], XC[1])
            xts[1] = xt1
            # second t1 copy rides the scalar queue (also warms it)
            nc.scalar.dma_start(tb[:, S:2 * S], CB[:, :])
            # t2 = [ss | cc] derived from t1 = [cc | -ss]
            nc.vector.tensor_copy(tb[:, T2O + HK:T2O + S], tb[:, 0:HK])
            nc.vector.tensor_scalar_mul(tb[:, T2O:T2O + HK], tb[:, HK:S],
                                        -1.0)
            nc.vector.tensor_copy(tb[:, T2O + S:4 * S], tb[:, T2O:T2O + S])

            def compute_single(n, split_mul1=False):
                xt = xts[n]
                pq = wpool.tile([128, 2 * S], f16, tag="PQ1", bufs=2,
                                name=f"PQ{n}")
                og = opool.tile([128, S], f16, tag="o1", bufs=2,
                                name=f"o{n}")
                if split_mul1:
                    nc.vector.tensor_mul(pq[:, 0:HK], xt[:, 0:HK],
                                         tb[:, 0:HK])
                    nc.vector.tensor_mul(pq[:, HK:S], xt[:, HK:S],
                                         tb[:, HK:S])
                else:
                    nc.vector.tensor_mul(pq[:, 0:S], xt[:, :], tb[:, 0:S])
                nc.vector.tensor_mul(pq[:, S:2 * S], xt[:, :],
                                     tb[:, T2O:T2O + S])
                avn = pq[:, :].rearrange("p (m e c) -> p m e c", m=2, e=2)
                ovn = og[:, :].rearrange("p (m c) -> p m c", m=2)
                nc.vector.tensor_add(ovn[:, :, :], avn[:, :, 0, :],
                                     avn[:, :, 1, :])
                nc.scalar.dma_start(OUT[n], og[:, :])

            compute_single(0, split_mul1=True)
            compute_single(1)

            # pairs with plain-2D muls against doubled tables
            for n0 in (2, 4, 6):
                xg = xpool.tile([128, 2 * S], f16, tag="x2", bufs=2,
                                name=f"xg{n0}")
                nc.sync.dma_start(
                    xg[:, :].rearrange("p (nl col) -> p nl col", nl=2),
                    XC[n0:n0 + 2].rearrange("nl p col -> p nl col"))
                pq = wpool.tile([128, 4 * S], f16, tag="PQ2", bufs=2,
                                name=f"PQ{n0}")
                og = opool.tile([128, 2 * S], f16, tag="o2", bufs=2,
                                name=f"o{n0}")
                nc.vector.tensor_mul(pq[:, 0:2 * S], xg[:, :],
                                     tb[:, 0:2 * S])
                nc.vector.tensor_mul(pq[:, 2 * S:4 * S], xg[:, :],
                                     tb[:, T2O:4 * S])
                # pq col = (m, nl, e, c); og col = (m, nl, c)
                a4 = pq[:, :].rearrange("p (m nl e c) -> p m nl e c",
                                        m=2, nl=2, e=2)
                ov = og[:, :].rearrange("p (m nl c) -> p m nl c",
                                        m=2, nl=2)
                o4 = og[:, :].rearrange("p (m nl c) -> p m nl c",
                                        m=2, nl=2)
                if n0 == 6:
                    # split the last pair so the out stream drains early
                    nc.vector.tensor_add(ov[:, 0, :, :], a4[:, 0, :, 0, :],
                                         a4[:, 0, :, 1, :])
                    nc.sync.dma_start(OUT[6][:, 0:HK], og[:, 0:HK])
                    nc.scalar.dma_start(OUT[7][:, 0:HK], og[:, HK:S])
                    nc.vector.tensor_add(ov[:, 1, :, :], a4[:, 1, :, 0, :],
                                         a4[:, 1, :, 1, :])
                    nc.sync.dma_start(OUT[6][:, HK:S], og[:, S:S + HK])
                    nc.scalar.dma_start(OUT[7][:, HK:S], og[:, S + HK:2 * S])
                else:
                    nc.vector.tensor_add(ov[:, :, :, :], a4[:, :, :, 0, :],
                                         a4[:, :, :, 1, :])
                    for nl in range(2):
                        eng = nc.sync if nl == 0 else nc.scalar
                        eng.dma_start(
                            OUT[n0 + nl].rearrange("p (m c) -> p m c", m=2),
                            o4[:, :, nl, :])
    nc.compile()
    return nc


def _get_built():
    if "nc" not in _cache:
        _cache["nc"] = _build_nc()
    return _cache["nc"]


def kernel(x, W, b):
    from concourse.bass_utils import run_bass_kernel_spmd

    assert x.shape == (B, S, H, D)
    xc = np.transpose(np.asarray(x, dtype=np.float32),
                      (0, 2, 1, 3)).reshape(N, S, D)
    # col (e, t, k) <- xc[n, 128t+p, 2k+e], fp16
    xs = np.ascontiguousarray(
        xc.reshape(N, NT, 128, K2, 2).transpose(0, 2, 4, 1, 3)
        .reshape(N, 128, S)).astype(np.float16)
    if "cb" not in _cache:
        _cache["cb"] = _make_tables()
    cbb = _cache["cb"]

    nc = _get_built()
    in_maps = []
    for c in range(NCORES):
        in_maps.append({
            "XC": np.ascontiguousarray(xs[NPC * c:NPC * (c + 1)]),
            "CB": cbb,
        })
    res = run_bass_kernel_spmd(nc, in_maps, core_ids=list(range(NCORES)))
    if res.exec_time_ns is not None:
        print(f"HW exec time: {res.exec_time_ns} ns")
    outs = np.concatenate([res.results[c]["OUT"] for c in range(NCORES)],
                          axis=0)                   # [N, 128, S]
    # og col = (m, t, k): s = 128t + p, d = 2k + m
    full = (outs.reshape(N, 128, 2, NT, K2).transpose(0, 3, 1, 4, 2)
            .reshape(N, S, D).astype(np.float32))
    full = full.reshape(B, H, S, D).transpose(0, 2, 1, 3)
    return np.ascontiguousarray(full)


# revision 19
# speedup vs baseline: 1.0080x; 1.0080x over previous
"""BetweennessRoPE Trainium2 kernel — fixed-table fp16 formulation.

Math (why no betweenness is computed on device):
  score = relu(1 - (path-direct)/max(direct,1e-6)) lies in [0,1] by the
  triangle inequality, so between in [0, 1/2046] and
  pos_adj = (between-0.5)*0.1 in [-0.05, -0.05+4.888e-5].  Hence for
  every position frac = 0.95 + delta with |delta| <= ~1.1e-4 (including
  the fp32 rounding of fl(s + pos_adj) at s ~ 2048).  The interpolated
  tables therefore differ from fixed-f tables
      C[s] = (1-f)*fcos[s-1] + f*fcos[s],  f = 0.95 + 0.05/2046
  by <= ~1.1e-4 * |fcos[s]-fcos[s-1]|, giving output error ~2e-4 of the
  output scale — far below the 2e-2 gate.  s=0 is exact (clip pins
  adj_pos to 0 and C[0]=fcos[0]).  So the kernel is a pure elementwise
  rotation with per-(s,k) constants:
      oe = xe*cc - xo*ss ;  oo = xo*cc + xe*ss.

Numerics: fp16 x / tables / products / outputs (DVE computes fp32
internally, rounds once on write) add ~1.5e-3 relative noise — still
~10x under the gate — and halve both DMA traffic and DVE cycles
(2x_1P packed mode needs 16-bit dense operands).

Layout: host de-interleaves even/odd features and converts to fp16.
Per slice [128, 2048]: partition p, col (e, t, k), s = 128t + p,
d = 2k + e.  Table t1 = [cc|-ss] is DMA'd (halves interleaved with the
first x slice so DVE starts ASAP); t2 = [ss|cc] is derived on-device.
Slices 0-1 run singly (slice 0 with split muls to start right after
the first 0.25 MiB lands); slices 2-7 run in pairs using stride-0
broadcast APs on the table so one TT covers both slices.  Per slice,
pq = [x*t1 | x*t2] then one fused add folds the e-halves into
og = [oe | oo].  GpSimd stays idle on purpose: concurrent Q7 TT ops
contend on the shared DVE/POOL SBUF port (measured 2.4-2.9x DVE
slowdown).  Input + table ride the SyncE DMA queue; output stores
alternate between the ScalarE and SyncE queues.
"""

import os
import numpy as np

B, S, H, D = 4, 2048, 16, 128
N = B * H
NCORES = 8
NPC = N // NCORES    # 8 slices per core
NT = S // 128        # 16
K2 = D // 2          # 64
HK = S // 2          # 1024 (cols per e-half)

_cache = {}


def _make_tables():
    base = (1.0 / (10000.0 ** (np.arange(0, D, 2, dtype=np.float32)
                               / np.float32(D)))).astype(np.float32)
    freqs = (np.arange(S, dtype=np.float32)[:, None]
             * base[None, :]).astype(np.float32)
    fcos = np.cos(freqs).astype(np.float32)
    fsin = np.sin(freqs).astype(np.float32)
    lo = np.maximum(np.arange(S) - 1, 0)
    f = 0.95 + 0.05 / 2046.0
    cc = ((1.0 - f) * fcos[lo].astype(np.float64)
          + f * fcos.astype(np.float64))
    ss = ((1.0 - f) * fsin[lo].astype(np.float64)
          + f * fsin.astype(np.float64))

    def blk(t):  # [S, 64] -> [128, NT*64], col (t, k)
        return t.reshape(NT, 128, K2).transpose(1, 0, 2).reshape(128, HK)

    t1 = np.concatenate([blk(cc), blk(-ss)], axis=1)       # [128, 2048]
    return np.ascontiguousarray(t1).astype(np.float16)


def _build_nc():
    import concourse.bacc as bacc
    import concourse.mybir as mybir
    from concourse.bass import broadcast_tensor_aps
    from concourse.tile import TileContext

    f16 = mybir.dt.float16

    nc = bacc.Bacc()
    XC = nc.dram_tensor("XC", [NPC, 128, S], f16, kind="ExternalInput")
    OUT = nc.dram_tensor("OUT", [NPC, 128, S], f16, kind="ExternalOutput")
    CB = nc.dram_tensor("CB", [128, S], f16, kind="ExternalInput")

    with TileContext(nc) as tc:
        with (
            tc.tile_pool(name="const", bufs=1) as cpool,
            tc.tile_pool(name="xbuf", bufs=4) as xpool,
            tc.tile_pool(name="obuf", bufs=4) as opool,
            tc.tile_pool(name="pq", bufs=2) as wpool,
        ):
            tb = cpool.tile([128, 2 * S], f16, tag="tb", name="tb")
            xts = {}

            def load_single(n):
                xt = xpool.tile([128, S], f16, tag="x1", bufs=4,
                                name=f"x{n}")
                nc.sync.dma_start(xt[:, :], XC[n])
                xts[n] = xt
                return xt

            # startup: interleave table halves with slice-0 halves so the
            # first mul only waits for 0.5 MiB of DMA
            x0 = xpool.tile([128, S], f16, tag="x1", bufs=4, name="x0")
            nc.sync.dma_start(tb[:, 0:HK], CB[:, 0:HK])          # cc
            nc.sync.dma_start(x0[:, 0:HK], XC[0][:, 0:HK])       # xe
            nc.sync.dma_start(tb[:, HK:S], CB[:, HK:S])          # -ss
            nc.sync.dma_start(x0[:, HK:S], XC[0][:, HK:S])       # xo
            xts[0] = x0
            load_single(1)
            # t2 = [ss | cc] derived from t1 = [cc | -ss]
            nc.vector.tensor_copy(tb[:, S + HK:2 * S], tb[:, 0:HK])
            nc.vector.tensor_scalar_mul(tb[:, S:S + HK], tb[:, HK:S], -1.0)

            def compute_single(n, split_mul1=False, tail_quarters=False):
                xt = xts[n]
                pq = wpool.tile([128, 2 * S], f16, tag="PQ1", bufs=3,
                                name=f"PQ{n}")
                og = opool.tile([128, S], f16, tag="o1", bufs=4,
                                name=f"o{n}")
                if split_mul1:
                    nc.vector.tensor_mul(pq[:, 0:HK], xt[:, 0:HK],
                                         tb[:, 0:HK])
                    nc.vector.tensor_mul(pq[:, HK:S], xt[:, HK:S],
                                         tb[:, HK:S])
                else:
                    nc.vector.tensor_mul(pq[:, 0:S], xt[:, :], tb[:, 0:S])
                nc.vector.tensor_mul(pq[:, S:2 * S], xt[:, :], tb[:, S:2 * S])
                avn = pq[:, :].rearrange("p (m e c) -> p m e c", m=2, e=2)
                ovn = og[:, :].rearrange("p (m c) -> p m c", m=2)
                if tail_quarters:
                    qk = HK // 2
                    for j in range(4):
                        m, h = j // 2, (j % 2) * qk
                        nc.vector.tensor_add(
                            ovn[:, m, h:h + qk], avn[:, m, 0, h:h + qk],
                            avn[:, m, 1, h:h + qk])
                        eng = nc.scalar if j % 2 == 0 else nc.sync
                        col = m * HK + h
                        eng.dma_start(OUT[n][:, col:col + qk],
                                      og[:, col:col + qk])
                else:
                    nc.vector.tensor_add(ovn[:, :, :], avn[:, :, 0, :],
                                         avn[:, :, 1, :])
                    nc.scalar.dma_start(OUT[n], og[:, :])

            compute_single(0, split_mul1=True)
            compute_single(1)
            for n in range(2, NPC - 1):
                load_single(n)
                compute_single(n)
            load_single(NPC - 1)
            compute_single(NPC - 1, tail_quarters=True)
    nc.compile()
    return nc


def _get_built():
    if "nc" not in _cache:
        _cache["nc"] = _build_nc()
    return _cache["nc"]


def kernel(x, W, b):
    from concourse.bass_utils import run_bass_kernel_spmd

    assert x.shape == (B, S, H, D)
    xc = np.transpose(np.asarray(x, dtype=np.float32),
                      (0, 2, 1, 3)).reshape(N, S, D)
    # col (e, t, k) <- xc[n, 128t+p, 2k+e], fp16
    xs = np.ascontiguousarray(
        xc.reshape(N, NT, 128, K2, 2).transpose(0, 2, 4, 1, 3)
        .reshape(N, 128, S)).astype(np.float16)
    if "cb" not in _cache:
        _cache["cb"] = _make_tables()
    cbb = _cache["cb"]

    nc = _get_built()
    in_maps = []
    for c in range(NCORES):
        in_maps.append({
            "XC": np.ascontiguousarray(xs[NPC * c:NPC * (c + 1)]),
            "CB": cbb,
        })
    res = run_bass_kernel_spmd(nc, in_maps, core_ids=list(range(NCORES)))
    if res.exec_time_ns is not None:
        print(f"HW exec time: {res.exec_time_ns} ns")
    outs = np.concatenate([res.results[c]["OUT"] for c in range(NCORES)],
                          axis=0)                   # [N, 128, S]
    # og col = (m, t, k): s = 128t + p, d = 2k + m
    full = (outs.reshape(N, 128, 2, NT, K2).transpose(0, 3, 1, 4, 2)
            .reshape(N, S, D).astype(np.float32))
    full = full.reshape(B, H, S, D).transpose(0, 2, 1, 3)
    return np.ascontiguousarray(full)
